# revision 5
# baseline (speedup 1.0000x reference)
"""MQA self-attention kernel for Trainium2, 8 NeuronCores.

Reference computation (fp32):
    q = x @ wq.T + bq        -> [B,S,1024] -> heads via (hidden num_heads) split
    k = x @ wk.T + bk        -> [B,S,64]  (single shared KV head)
    v = x @ wv.T + bv
    scores = q @ k.T / 8 ; attn = softmax(scores) ; h = attn @ v
    out = merge_heads(h) @ wo.T + bo

Sharding (8 cores, no collectives): core c handles batch b=c//4 and head
group g=c%4 (4 of the 16 q-heads).  The shared K/V head is replicated.
Each core returns the partial output h_g @ wo_g.T [S, D]; the host sums
the 4 head-group partials per batch and adds the bias terms.

Per-core schedule (PE-bound at ~280k cycles):
 - Projections in fp32r (full PE rate at N=512).  K lands in psum rows
   0:64 (wvkT = [wk.T | wv.T]) so its evac needs no partition shift; the
   KT row block is duplicated to partitions 64:128 so odd heads' scores
   matmuls can run against QT stored in the other partition half.
 - scoresT[sk,sq] = KT.T @ QT per head in [128,1024] psum tiles.
 - exp is split across two engines (per key tile): ACT runs the real
   Exp activation (bf16 out); DVE computes a Schraudolph-style exp2 via
   one tensor_scalar (x*A+B -> int16, bitcast bf16).  Softmax
   renormalizes, so the ~3% sawtooth error mostly cancels; measured
   end-to-end rel err ~1e-2 vs the 2e-2 gate.
 - PV is flipped: the exp tile [sk 128, sq 128] is the stationary
   operand and V' = [V | 1] [128, 65] moves, so each matmul costs 65
   moving rows instead of 512 (PE charges by moving dim only).  The
   ones column accumulates the softmax denominator in the same psum
   tile ([sq, 65], col 64 = sum).  The 4 interleaved regions share one
   2KB zero region: start/stop bracket the whole bank's group.
 - normalize on evac: DVE reciprocal of the strided denominators, then
   DVE tensor_scalar / ACT Copy-with-scale write normalized bf16 h for
   a head pair packed [sq, h_even|h_odd].
 - DMA-transpose (free XBAR engine) flips each [sq 128, 128] pair block
   into ht [128 qd, sq] for the out projection (bf16, fp32 accum).
 - block 0's out projection is interleaved into block 1's first head so
   the PE never waits on the transpose latency.
"""

import numpy as np

NUM_HEADS = 16
Dh = 64
B, S, D = 2, 2048, 1024
G = 4            # head groups (cores per batch)
HG = 4           # heads per group
QD = HG * Dh     # 256 local q dims
NK = D // 128    # 8 contraction tiles for projections
NSK = S // 128   # 16 key tiles
W = 512          # matmul moving width
NB = 2           # sq blocks of 1024
BLK = 1024
N_CORES = 8

# Schraudolph exp2 constants (bf16 bit domain), score scale 1/8 folded in.
EXP_A = float(128.0 / np.log(2.0) * 0.125)
EXP_B = float(127.0 * 128.0 - 128.0 * np.log2(1.03279))

# exp engine split per key tile: these sk go to DVE (schraudolph).
DVE_SK = frozenset({2, 5, 8, 11, 13, 14})
# normalize-mul engine split per sq-tile: these go to ACT Copy-with-scale.
ACT_MUL = frozenset({1, 5})

_CACHE = {}


def _build_nc():
    from contextlib import ExitStack

    import concourse.bass as bass
    import concourse.mybir as mybir
    import concourse.tile as tile
    from concourse import bacc
    from concourse.masks import make_identity

    F32 = mybir.dt.float32
    F32R = mybir.dt.float32r
    BF16 = mybir.dt.bfloat16
    I16 = mybir.dt.int16
    EXP = mybir.ActivationFunctionType.Exp
    COPY = mybir.ActivationFunctionType.Copy
    MUL = mybir.AluOpType.mult
    ADD = mybir.AluOpType.add

    nc = bacc.Bacc("TRN2", target_bir_lowering=False, debug=False)

    xT = nc.declare_dram_parameter("xT", [D, S], F32R, isOutput=False)
    wqT = nc.declare_dram_parameter("wqT", [D, QD], F32R, isOutput=False)
    wvkT = nc.declare_dram_parameter("wvkT", [D, 128], F32R, isOutput=False)
    woT = nc.declare_dram_parameter("woT", [QD, D], BF16, isOutput=False)
    bqp = nc.declare_dram_parameter("bq", [128, 2], F32, isOutput=False)
    part = nc.declare_dram_parameter("part", [S, D], F32, isOutput=True)

    with tile.TileContext(nc) as tc, ExitStack() as ctx:
        const = ctx.enter_context(tc.tile_pool(name="const", bufs=1))
        persist = ctx.enter_context(tc.tile_pool(name="persist", bufs=1))

        wq_sb = const.tile([128, NK * QD], F32R)    # ktile kt at cols [kt*QD:+QD]
        wvk_sb = const.tile([128, NK * 128], F32R)  # cols 0:64 = wkT, 64:128 = wvT
        wo_sb = const.tile([128, 2 * D], BF16)      # qd-ktile p at cols [p*D:+D]
        bq_sb = const.tile([128, 2], F32)
        ident = const.tile([128, 128], F32)

        kt_sb = persist.tile([128, S], F32R)        # KT dup in both partition halves
        qt_sb = persist.tile([128, 2 * S], F32R)    # pass p: heads (2p,2p+1) stacked
        v1_sb = persist.tile([128, NSK * 65], BF16)  # V' tile sk at cols [sk*65:+65]
        ht_sb = persist.tile([128, 2 * BLK], BF16)  # block-local hT, pair p at [p*BLK:+BLK]

        make_identity(nc, ident[:])
        nc.vector.memset(v1_sb[:, 64:NSK * 65:65], 1.0)

        # ---- Phase 0/1: DMAs + projections ----------------------------
        with tc.tile_pool(name="xp", bufs=1) as xp:
            x_sb = xp.tile([128, NK * S], F32R)     # xT ktile kt at cols [kt*S:+S]
            vt_sb = xp.tile([128, S], F32)          # VT in rows 64:128

            # per-kt bundles (weights + x half-tiles) alternate SP/Pool queues
            for kt in range(NK):
                eng = nc.sync if kt % 2 == 0 else nc.gpsimd
                eng.dma_start(wvk_sb[:, kt * 128:(kt + 1) * 128], wvkT[kt * 128:(kt + 1) * 128, :])
                eng.dma_start(wq_sb[:, kt * QD:(kt + 1) * QD], wqT[kt * 128:(kt + 1) * 128, :])
                for h in range(2):
                    eng.dma_start(
                        x_sb[:, kt * S + h * BLK: kt * S + (h + 1) * BLK],
                        xT[kt * 128:(kt + 1) * 128, h * BLK:(h + 1) * BLK],
                    )
            nc.sync.dma_start(bq_sb[:], bqp[:, :])
            for p in range(2):
                nc.sync.dma_start(wo_sb[:, p * D:(p + 1) * D], woT[p * 128:(p + 1) * 128, :])

            with (
                tc.tile_pool(name="vkps", bufs=1, space="PSUM") as vkps,
                tc.tile_pool(name="qps", bufs=1, space="PSUM") as qps,
            ):
                vk_ps = [vkps.tile([128, BLK], F32, name=f"vk{h}") for h in range(2)]
                q_ps = [qps.tile([128, BLK], F32, name=f"q{h}") for h in range(2)]
                for kt in range(NK):
                    for h in range(2):
                        nc.tensor.matmul(
                            vk_ps[h][:, 0:W],
                            lhsT=wvk_sb[:, kt * 128:(kt + 1) * 128],
                            rhs=x_sb[:, kt * S + h * BLK: kt * S + h * BLK + W],
                            start=(kt == 0), stop=(kt == NK - 1),
                        )
                        nc.tensor.matmul(
                            vk_ps[h][:, W:BLK],
                            lhsT=wvk_sb[:, kt * 128:(kt + 1) * 128],
                            rhs=x_sb[:, kt * S + h * BLK + W: kt * S + (h + 1) * BLK],
                            start=(kt == 0), stop=(kt == NK - 1),
                        )
                        for n in range(2):
                            nc.tensor.matmul(
                                q_ps[h][:, n * W:(n + 1) * W],
                                lhsT=wq_sb[:, kt * QD: kt * QD + 128],
                                rhs=x_sb[:, kt * S + h * BLK + n * W: kt * S + h * BLK + (n + 1) * W],
                                start=(kt == 0), stop=(kt == NK - 1),
                            )
                # evacs: KT + VT on ACT, Q pass0 (+bias) on DVE
                for h in range(2):
                    nc.scalar.copy(kt_sb[0:64, h * BLK:(h + 1) * BLK], vk_ps[h][0:64, :])
                    nc.scalar.copy(vt_sb[64:128, h * BLK:(h + 1) * BLK], vk_ps[h][64:128, :])
                    nc.vector.tensor_scalar_add(
                        qt_sb[:, h * BLK:(h + 1) * BLK], q_ps[h][:, :], bq_sb[:, 0:1]
                    )
                nc.gpsimd.tensor_copy(kt_sb[64:128, :], kt_sb[0:64, :])

            # Q pass 1 (heads 2,3) reuses freed banks; V' transposes after
            with (
                tc.tile_pool(name="qps2", bufs=1, space="PSUM") as qps2,
                tc.tile_pool(name="trps", bufs=2, space="PSUM") as trps,
            ):
                q_ps2 = [qps2.tile([128, BLK], F32, name=f"q2{h}") for h in range(2)]
                for kt in range(NK):
                    for h in range(2):
                        for n in range(2):
                            nc.tensor.matmul(
                                q_ps2[h][:, n * W:(n + 1) * W],
                                lhsT=wq_sb[:, kt * QD + 128: kt * QD + 256],
                                rhs=x_sb[:, kt * S + h * BLK + n * W: kt * S + h * BLK + (n + 1) * W],
                                start=(kt == 0), stop=(kt == NK - 1),
                            )
                for sk in range(NSK):
                    tr_ps = trps.tile([128, Dh], F32, name="trp")
                    nc.tensor.transpose(
                        tr_ps[:], vt_sb[64:128, sk * 128:(sk + 1) * 128],
                        ident[64:128, 64:128],
                    )
                    nc.vector.tensor_copy(v1_sb[:, sk * 65: sk * 65 + 64], tr_ps[:])
                for h in range(2):
                    nc.vector.tensor_scalar_add(
                        qt_sb[:, S + h * BLK: S + (h + 1) * BLK], q_ps2[h][:, :], bq_sb[:, 1:2]
                    )

        # ---- Phase 2: attention + output projection --------------------
        with (
            tc.tile_pool(name="expp", bufs=4) as expp,
            tc.tile_pool(name="scps", bufs=2, space="PSUM") as scps,
            tc.tile_pool(name="hps", bufs=1, space="PSUM") as hps,
            tc.tile_pool(name="outps", bufs=2, space="PSUM") as outps,
            tc.tile_pool(name="smalls", bufs=2) as smalls,
            tc.tile_pool(name="hnp", bufs=2) as hnp,
            tc.tile_pool(name="osbp", bufs=3) as osbp,
        ):
            h_ps = [hps.tile([128, W], F32, name=f"hb{i}") for i in range(2)]

            def emit_out_chunk(blk, s):
                # one s-chunk of the block's output projection: 2 n-halves
                for n in range(2):
                    o_ps = outps.tile([128, W], F32, name="ops")
                    for p in range(2):
                        nc.tensor.matmul(
                            o_ps[:],
                            lhsT=ht_sb[:, p * BLK + s * 128: p * BLK + (s + 1) * 128],
                            rhs=wo_sb[:, p * D + n * W: p * D + (n + 1) * W],
                            start=(p == 0), stop=(p == 1),
                        )
                    o_sb = osbp.tile([128, W], F32, name="osb")
                    if (s + n) % 2 == 0:
                        nc.scalar.copy(o_sb[:], o_ps[:])
                    else:
                        nc.vector.tensor_copy(o_sb[:], o_ps[:])
                    nc.gpsimd.dma_start(
                        part[blk * BLK + s * 128: blk * BLK + (s + 1) * 128,
                             n * W:(n + 1) * W],
                        o_sb[:],
                    )

            for blk in range(NB):
                hn_t = None
                for hl in range(HG):
                    half, pair = hl % 2, hl // 2
                    rows = slice(0, 64) if half == 0 else slice(64, 128)
                    qcol = pair * S + blk * BLK
                    exp_tiles = [None] * NSK

                    def emit_pv(sk):
                        et = exp_tiles[sk]
                        for t in range(8):
                            c = t % 4
                            nc.tensor.matmul(
                                h_ps[t // 4][:, c * 65: c * 65 + 65],
                                lhsT=et[:, t * 128:(t + 1) * 128],
                                rhs=v1_sb[:, sk * 65:(sk + 1) * 65],
                                start=(sk == 0 and c == 0),
                                stop=(sk == NSK - 1 and c == 3),
                            )

                    for sk in range(NSK):
                        sc = scps.tile([128, BLK], F32, name="sc")
                        for n in range(2):
                            nc.tensor.matmul(
                                sc[:, n * W:(n + 1) * W],
                                lhsT=kt_sb[rows, sk * 128:(sk + 1) * 128],
                                rhs=qt_sb[rows, qcol + n * W: qcol + (n + 1) * W],
                                start=True, stop=True,
                            )
                        # interleave the previous block's out projection into
                        # this block's first head so PE never waits on the
                        # ht transposes at the block boundary
                        if blk == 1 and hl == 0 and sk < 8:
                            emit_out_chunk(0, sk)
                        et = expp.tile([128, BLK], BF16, name="et")
                        if sk in DVE_SK:
                            nc.vector.tensor_scalar(
                                et[:].bitcast(I16), sc[:], EXP_A, EXP_B, MUL, ADD
                            )
                        else:
                            nc.scalar.activation(et[:], sc[:], EXP, scale=0.125)
                        exp_tiles[sk] = et
                        if sk >= 2:
                            emit_pv(sk - 2)
                    emit_pv(NSK - 2)
                    emit_pv(NSK - 1)

                    # normalize + pack head pair [sq, h_even|h_odd] in bf16
                    if half == 0:
                        hn_t = hnp.tile([128, 8 * 128], BF16, name="hn")
                    rec = smalls.tile([128, 8], F32, name="rec")
                    for i in range(2):
                        nc.vector.reciprocal(
                            rec[:, i * 4:(i + 1) * 4], h_ps[i][:, 64:260:65]
                        )
                    for t in range(8):
                        i, c = t // 4, t % 4
                        dst = hn_t[:, t * 128 + half * 64: t * 128 + half * 64 + 64]
                        if t in ACT_MUL:
                            nc.scalar.activation(
                                dst, h_ps[i][:, c * 65: c * 65 + 64],
                                COPY, scale=rec[:, t:t + 1],
                            )
                        else:
                            nc.vector.tensor_scalar(
                                dst, h_ps[i][:, c * 65: c * 65 + 64],
                                rec[:, t:t + 1], None, MUL,
                            )
                    if half == 1:
                        for t in range(8):
                            nc.sync.dma_start_transpose(
                                ht_sb[:, pair * BLK + t * 128: pair * BLK + (t + 1) * 128],
                                hn_t[:, t * 128:(t + 1) * 128],
                            )

                if blk == 1:
                    for s in range(8):
                        emit_out_chunk(1, s)
                # block 0's out projection is emitted inside block 1 head 0

    nc.finalize()
    return nc


def _get_nc():
    if "nc" not in _CACHE:
        _CACHE["nc"] = _build_nc()
    return _CACHE["nc"]


def _to_bf16_bits(a):
    """fp32 -> bf16 bits (round-to-nearest-even), as uint16."""
    u = np.asarray(a, np.float32).view(np.uint32)
    return ((u + 0x7FFF + ((u >> 16) & 1)) >> 16).astype(np.uint16)


def _prep_core_inputs(inputs, wq, bq, wk, wv, wo):
    """Host-side shard prep: per-core transposed/rearranged operands."""
    xT = [np.ascontiguousarray(np.asarray(inputs[b], np.float32).T) for b in range(B)]
    wq3 = np.asarray(wq, np.float32).reshape(Dh, NUM_HEADS, D)
    bq2 = np.asarray(bq, np.float32).reshape(Dh, NUM_HEADS)
    wvkT = np.ascontiguousarray(
        np.concatenate([np.asarray(wk, np.float32).T, np.asarray(wv, np.float32).T], axis=1)
    )  # [1024, 128], K first
    wo_ = np.asarray(wo, np.float32)

    in_maps = []
    for c in range(N_CORES):
        b, g = divmod(c, G)
        heads = [g * HG + hl for hl in range(HG)]
        wqT_g = np.ascontiguousarray(
            np.concatenate([wq3[:, h, :].T for h in heads], axis=1)
        )  # [1024, 256]
        bq_g = np.ascontiguousarray(
            np.concatenate([bq2[:, h] for h in heads]).reshape(2, 128).T
        )  # [128, 2]: col p = heads (2p, 2p+1) stacked
        woT_g = _to_bf16_bits(wo_[:, g * QD:(g + 1) * QD].T)  # [256, 1024] bf16
        in_maps.append({
            "xT": xT[b],
            "wqT": wqT_g,
            "wvkT": wvkT,
            "woT": woT_g,
            "bq": bq_g,
        })
    return in_maps


def kernel(inputs, wq, bq, wk, bk, wv, bv, wo, bo):
    from concourse.bass_utils import run_bass_kernel_spmd

    nc = _get_nc()
    in_maps = _prep_core_inputs(inputs, wq, bq, wk, wv, wo)
    res = run_bass_kernel_spmd(nc, in_maps, list(range(N_CORES))).results

    wo_ = np.asarray(wo, np.float32)
    bias = (
        np.asarray(bo, np.float32)
        + wo_ @ np.tile(np.asarray(bv, np.float32), NUM_HEADS)
    )
    out = np.empty((B, S, D), np.float32)
    for b in range(B):
        acc = res[b * G]["part"].astype(np.float32).copy()
        for g in range(1, G):
            acc += res[b * G + g]["part"]
        out[b] = acc + bias
    return out


# revision 9
# speedup vs baseline: 1.1468x; 1.1468x over previous
"""MQA self-attention kernel for Trainium2, 8 NeuronCores.

Reference computation (fp32):
    q = x @ wq.T + bq        -> [B,S,1024] -> heads via (hidden num_heads) split
    k = x @ wk.T + bk        -> [B,S,64]  (single shared KV head)
    v = x @ wv.T + bv
    scores = q @ k.T / 8 ; attn = softmax(scores) ; h = attn @ v
    out = merge_heads(h) @ wo.T + bo

Sharding (8 cores, no collectives): core c handles batch b=c//4 and head
group g=c%4 (4 of the 16 q-heads).  The shared K/V head is replicated.
Each core returns the partial output h_g @ wo_g.T [S, D]; the host sums
the 4 head-group partials per batch and adds the bias terms.

Per-core schedule (PE-bound at ~280k cycles):
 - Projections in fp32r (full PE rate at N=512).  K lands in psum rows
   0:64 (wvkT = [wk.T | wv.T]) so its evac needs no partition shift; the
   KT row block is duplicated to partitions 64:128 so odd heads' scores
   matmuls can run against QT stored in the other partition half.
 - scoresT[sk,sq] = KT.T @ QT per head in [128,1024] psum tiles.
 - exp is split across two engines (per key tile): ACT runs the real
   Exp activation (bf16 out); DVE computes a Schraudolph-style exp2 via
   one tensor_scalar (x*A+B -> int16, bitcast bf16).  Softmax
   renormalizes, so the ~3% sawtooth error mostly cancels; measured
   end-to-end rel err ~1e-2 vs the 2e-2 gate.
 - PV is flipped: the exp tile [sk 128, sq 128] is the stationary
   operand and V' = [V | 1] [128, 65] moves, so each matmul costs 65
   moving rows instead of 512 (PE charges by moving dim only).  The
   ones column accumulates the softmax denominator in the same psum
   tile ([sq, 65], col 64 = sum).  The 4 interleaved regions share one
   2KB zero region: start/stop bracket the whole bank's group.
 - normalize on evac: DVE reciprocal of the strided denominators, then
   DVE tensor_scalar / ACT Copy-with-scale write normalized bf16 h for
   a head pair packed [sq, h_even|h_odd].
 - DMA-transpose (free XBAR engine) flips each [sq 128, 128] pair block
   into ht [128 qd, sq] for the out projection (bf16, fp32 accum).
 - block 0's out projection is interleaved into block 1's first head so
   the PE never waits on the transpose latency.
"""

import numpy as np

NUM_HEADS = 16
Dh = 64
B, S, D = 2, 2048, 1024
G = 4            # head groups (cores per batch)
HG = 4           # heads per group
QD = HG * Dh     # 256 local q dims
NK = D // 128    # 8 contraction tiles for projections
NSK = S // 128   # 16 key tiles
W = 512          # matmul moving width
NB = 2           # sq blocks of 1024
BLK = 1024
N_CORES = 8

# Schraudolph exp2 constants (bf16 bit domain), score scale 1/8 folded in.
EXP_A = float(128.0 / np.log(2.0) * 0.125)
EXP_B = float(127.0 * 128.0 - 128.0 * np.log2(1.03279))

# exp engine split per (sk, half): 13 of 32 half-tiles go to DVE
# (schraudolph), spread evenly so ACT/DVE ping-pong without starving PE.
DVE_HALF = frozenset(i for i in range(32) if (i * 13) % 32 < 13)
# normalize-mul engine split per sq-tile: these go to ACT Copy-with-scale.
ACT_MUL = frozenset({1, 5})

_CACHE = {}


def _build_nc():
    from contextlib import ExitStack

    import concourse.bass as bass
    import concourse.mybir as mybir
    import concourse.tile as tile
    from concourse import bacc
    from concourse.masks import make_identity

    F32 = mybir.dt.float32
    F32R = mybir.dt.float32r
    BF16 = mybir.dt.bfloat16
    I16 = mybir.dt.int16
    EXP = mybir.ActivationFunctionType.Exp
    COPY = mybir.ActivationFunctionType.Copy
    MUL = mybir.AluOpType.mult
    ADD = mybir.AluOpType.add

    nc = bacc.Bacc("TRN2", target_bir_lowering=False, debug=False)

    xT = nc.declare_dram_parameter("xT", [D, S], F32R, isOutput=False)
    wqT = nc.declare_dram_parameter("wqT", [D, QD], F32R, isOutput=False)
    wvkT = nc.declare_dram_parameter("wvkT", [D, 128], F32R, isOutput=False)
    woT = nc.declare_dram_parameter("woT", [QD, D], BF16, isOutput=False)
    bqp = nc.declare_dram_parameter("bq", [128, 2], F32, isOutput=False)
    part = nc.declare_dram_parameter("part", [S, D], F32, isOutput=True)

    with tile.TileContext(nc) as tc, ExitStack() as ctx:
        const = ctx.enter_context(tc.tile_pool(name="const", bufs=1))
        persist = ctx.enter_context(tc.tile_pool(name="persist", bufs=1))

        wq_sb = const.tile([128, NK * QD], F32R)    # ktile kt at cols [kt*QD:+QD]
        wvk_sb = const.tile([128, NK * 128], F32R)  # cols 0:64 = wkT, 64:128 = wvT
        wo_sb = const.tile([128, 2 * D], BF16)      # qd-ktile p at cols [p*D:+D]
        bq_sb = const.tile([128, 2], F32)
        ident = const.tile([128, 128], F32)

        kt_sb = persist.tile([128, S], F32R)        # KT dup in both partition halves
        qt_sb = persist.tile([128, 2 * S], F32R)    # pass p: heads (2p,2p+1) stacked
        v1_sb = persist.tile([128, NSK * 65], BF16)  # V' tile sk at cols [sk*65:+65]
        ht_sb = persist.tile([128, 2 * BLK], BF16)  # block-local hT, pair p at [p*BLK:+BLK]

        make_identity(nc, ident[:])
        nc.vector.memset(v1_sb[:, 64:NSK * 65:65], 1.0)

        # ---- Phase 0/1: DMAs + projections ----------------------------
        with tc.tile_pool(name="xp", bufs=1) as xp:
            x_sb = xp.tile([128, NK * S], F32R)     # xT ktile kt at cols [kt*S:+S]
            vt_sb = xp.tile([128, S], F32)          # VT in rows 64:128

            # per-kt bundles (weights + x half-tiles) alternate SP/Pool queues
            for kt in range(NK):
                eng = nc.sync if kt % 2 == 0 else nc.gpsimd
                eng.dma_start(wvk_sb[:, kt * 128:(kt + 1) * 128], wvkT[kt * 128:(kt + 1) * 128, :])
                eng.dma_start(wq_sb[:, kt * QD:(kt + 1) * QD], wqT[kt * 128:(kt + 1) * 128, :])
                for h in range(2):
                    eng.dma_start(
                        x_sb[:, kt * S + h * BLK: kt * S + (h + 1) * BLK],
                        xT[kt * 128:(kt + 1) * 128, h * BLK:(h + 1) * BLK],
                    )
            nc.sync.dma_start(bq_sb[:], bqp[:, :])
            for p in range(2):
                nc.sync.dma_start(wo_sb[:, p * D:(p + 1) * D], woT[p * 128:(p + 1) * 128, :])

            with (
                tc.tile_pool(name="vkps", bufs=1, space="PSUM") as vkps,
                tc.tile_pool(name="qps", bufs=1, space="PSUM") as qps,
            ):
                vk_ps = [vkps.tile([128, BLK], F32, name=f"vk{h}") for h in range(2)]
                q_ps = [qps.tile([128, BLK], F32, name=f"q{h}") for h in range(2)]
                for kt in range(NK):
                    for h in range(2):
                        nc.tensor.matmul(
                            vk_ps[h][:, 0:W],
                            lhsT=wvk_sb[:, kt * 128:(kt + 1) * 128],
                            rhs=x_sb[:, kt * S + h * BLK: kt * S + h * BLK + W],
                            start=(kt == 0), stop=(kt == NK - 1),
                        )
                        nc.tensor.matmul(
                            vk_ps[h][:, W:BLK],
                            lhsT=wvk_sb[:, kt * 128:(kt + 1) * 128],
                            rhs=x_sb[:, kt * S + h * BLK + W: kt * S + (h + 1) * BLK],
                            start=(kt == 0), stop=(kt == NK - 1),
                        )
                        for n in range(2):
                            nc.tensor.matmul(
                                q_ps[h][:, n * W:(n + 1) * W],
                                lhsT=wq_sb[:, kt * QD: kt * QD + 128],
                                rhs=x_sb[:, kt * S + h * BLK + n * W: kt * S + h * BLK + (n + 1) * W],
                                start=(kt == 0), stop=(kt == NK - 1),
                            )
                # evacs: KT + VT on ACT, Q pass0 (+bias) on DVE
                for h in range(2):
                    nc.scalar.copy(kt_sb[0:64, h * BLK:(h + 1) * BLK], vk_ps[h][0:64, :])
                    nc.scalar.copy(vt_sb[64:128, h * BLK:(h + 1) * BLK], vk_ps[h][64:128, :])
                    nc.vector.tensor_scalar_add(
                        qt_sb[:, h * BLK:(h + 1) * BLK], q_ps[h][:, :], bq_sb[:, 0:1]
                    )
                nc.gpsimd.tensor_copy(kt_sb[64:128, :], kt_sb[0:64, :])

            # Q pass 1 (heads 2,3) reuses freed banks; V' transposes after
            with (
                tc.tile_pool(name="qps2", bufs=1, space="PSUM") as qps2,
                tc.tile_pool(name="trps", bufs=2, space="PSUM") as trps,
            ):
                q_ps2 = [qps2.tile([128, BLK], F32, name=f"q2{h}") for h in range(2)]
                for kt in range(NK):
                    for h in range(2):
                        for n in range(2):
                            nc.tensor.matmul(
                                q_ps2[h][:, n * W:(n + 1) * W],
                                lhsT=wq_sb[:, kt * QD + 128: kt * QD + 256],
                                rhs=x_sb[:, kt * S + h * BLK + n * W: kt * S + h * BLK + (n + 1) * W],
                                start=(kt == 0), stop=(kt == NK - 1),
                            )
                for sk in range(NSK):
                    tr_ps = trps.tile([128, Dh], F32, name="trp")
                    nc.tensor.transpose(
                        tr_ps[:], vt_sb[64:128, sk * 128:(sk + 1) * 128],
                        ident[64:128, 64:128],
                    )
                    nc.vector.tensor_copy(v1_sb[:, sk * 65: sk * 65 + 64], tr_ps[:])
                for h in range(2):
                    nc.vector.tensor_scalar_add(
                        qt_sb[:, S + h * BLK: S + (h + 1) * BLK], q_ps2[h][:, :], bq_sb[:, 1:2]
                    )

        # ---- Phase 2: attention + output projection --------------------
        with (
            tc.tile_pool(name="expp", bufs=8) as expp,
            tc.tile_pool(name="scps", bufs=5, space="PSUM") as scps,
            tc.tile_pool(name="hps", bufs=1, space="PSUM") as hps,
            tc.tile_pool(name="outps", bufs=1, space="PSUM") as outps,
            tc.tile_pool(name="smalls", bufs=2) as smalls,
            tc.tile_pool(name="hnp", bufs=2) as hnp,
            tc.tile_pool(name="osbp", bufs=3) as osbp,
        ):
            h_ps = [hps.tile([128, W], F32, name=f"hb{i}") for i in range(2)]

            def emit_out_chunk(blk, s, tail=False):
                # one s-chunk of the block's output projection: 2 n-halves.
                # The tail (nothing left to interleave with) borrows sc-pool
                # banks so several chunks can be in flight at once.
                for n in range(2):
                    if tail:
                        o_ps = scps.tile([128, W], F32, name="sc")
                    else:
                        o_ps = outps.tile([128, W], F32, name="ops")
                    for p in range(2):
                        nc.tensor.matmul(
                            o_ps[:],
                            lhsT=ht_sb[:, p * BLK + s * 128: p * BLK + (s + 1) * 128],
                            rhs=wo_sb[:, p * D + n * W: p * D + (n + 1) * W],
                            start=(p == 0), stop=(p == 1),
                        )
                    o_sb = osbp.tile([128, W], F32, name="osb")
                    if (s + n) % 2 == 0:
                        nc.scalar.copy(o_sb[:], o_ps[:])
                    else:
                        nc.vector.tensor_copy(o_sb[:], o_ps[:])
                    nc.gpsimd.dma_start(
                        part[blk * BLK + s * 128: blk * BLK + (s + 1) * 128,
                             n * W:(n + 1) * W],
                        o_sb[:],
                    )

            for blk in range(NB):
                hn_t = None
                for hl in range(HG):
                    half, pair = hl % 2, hl // 2
                    rows = slice(0, 64) if half == 0 else slice(64, 128)
                    qcol = pair * S + blk * BLK
                    exp_tiles = [None] * NSK

                    def emit_pv(sk):
                        et0, et1 = exp_tiles[sk]
                        for t in range(8):
                            et = et0 if t < 4 else et1
                            c = t % 4
                            nc.tensor.matmul(
                                h_ps[t // 4][:, c * 65: c * 65 + 65],
                                lhsT=et[:, c * 128:(c + 1) * 128],
                                rhs=v1_sb[:, sk * 65:(sk + 1) * 65],
                                start=(sk == 0 and c == 0),
                                stop=(sk == NSK - 1 and c == 3),
                            )

                    for sk in range(NSK):
                        ets = []
                        for n in range(2):
                            sc = scps.tile([128, W], F32, name="sc")
                            nc.tensor.matmul(
                                sc[:],
                                lhsT=kt_sb[rows, sk * 128:(sk + 1) * 128],
                                rhs=qt_sb[rows, qcol + n * W: qcol + (n + 1) * W],
                                start=True, stop=True,
                            )
                            et = expp.tile([128, W], BF16, name="et")
                            if sk * 2 + n in DVE_HALF:
                                nc.vector.tensor_scalar(
                                    et[:].bitcast(I16), sc[:], EXP_A, EXP_B, MUL, ADD
                                )
                            else:
                                nc.scalar.activation(et[:], sc[:], EXP, scale=0.125)
                            ets.append(et)
                        exp_tiles[sk] = ets
                        # interleave the previous block's out projection into
                        # this block's first head so PE never waits on the
                        # ht transposes at the block boundary
                        if blk == 1 and hl == 0 and sk < 8:
                            emit_out_chunk(0, sk)
                        if sk >= 2:
                            emit_pv(sk - 2)
                    emit_pv(NSK - 2)
                    emit_pv(NSK - 1)

                    # normalize + pack head pair [sq, h_even|h_odd] in bf16
                    if half == 0:
                        hn_t = hnp.tile([128, 8 * 128], BF16, name="hn")
                    rec = smalls.tile([128, 8], F32, name="rec")
                    for i in range(2):
                        nc.vector.reciprocal(
                            rec[:, i * 4:(i + 1) * 4], h_ps[i][:, 64:260:65]
                        )
                    for t in range(8):
                        i, c = t // 4, t % 4
                        dst = hn_t[:, t * 128 + half * 64: t * 128 + half * 64 + 64]
                        if t in ACT_MUL:
                            nc.scalar.activation(
                                dst, h_ps[i][:, c * 65: c * 65 + 64],
                                COPY, scale=rec[:, t:t + 1],
                            )
                        else:
                            nc.vector.tensor_scalar(
                                dst, h_ps[i][:, c * 65: c * 65 + 64],
                                rec[:, t:t + 1], None, MUL,
                            )
                    if half == 1:
                        for t in range(8):
                            nc.sync.dma_start_transpose(
                                ht_sb[:, pair * BLK + t * 128: pair * BLK + (t + 1) * 128],
                                hn_t[:, t * 128:(t + 1) * 128],
                            )

                if blk == 1:
                    for s in range(8):
                        emit_out_chunk(1, s, tail=True)
                # block 0's out projection is emitted inside block 1 head 0

    nc.finalize()
    return nc


def _get_nc():
    if "nc" not in _CACHE:
        _CACHE["nc"] = _build_nc()
    return _CACHE["nc"]


def _to_bf16_bits(a):
    """fp32 -> bf16 bits (round-to-nearest-even), as uint16."""
    u = np.asarray(a, np.float32).view(np.uint32)
    return ((u + 0x7FFF + ((u >> 16) & 1)) >> 16).astype(np.uint16)


def _prep_core_inputs(inputs, wq, bq, wk, wv, wo):
    """Host-side shard prep: per-core transposed/rearranged operands."""
    xT = [np.ascontiguousarray(np.asarray(inputs[b], np.float32).T) for b in range(B)]
    wq3 = np.asarray(wq, np.float32).reshape(Dh, NUM_HEADS, D)
    bq2 = np.asarray(bq, np.float32).reshape(Dh, NUM_HEADS)
    wvkT = np.ascontiguousarray(
        np.concatenate([np.asarray(wk, np.float32).T, np.asarray(wv, np.float32).T], axis=1)
    )  # [1024, 128], K first
    wo_ = np.asarray(wo, np.float32)

    in_maps = []
    for c in range(N_CORES):
        b, g = divmod(c, G)
        heads = [g * HG + hl for hl in range(HG)]
        wqT_g = np.ascontiguousarray(
            np.concatenate([wq3[:, h, :].T for h in heads], axis=1)
        )  # [1024, 256]
        bq_g = np.ascontiguousarray(
            np.concatenate([bq2[:, h] for h in heads]).reshape(2, 128).T
        )  # [128, 2]: col p = heads (2p, 2p+1) stacked
        woT_g = _to_bf16_bits(wo_[:, g * QD:(g + 1) * QD].T)  # [256, 1024] bf16
        in_maps.append({
            "xT": xT[b],
            "wqT": wqT_g,
            "wvkT": wvkT,
            "woT": woT_g,
            "bq": bq_g,
        })
    return in_maps


def kernel(inputs, wq, bq, wk, bk, wv, bv, wo, bo):
    from concourse.bass_utils import run_bass_kernel_spmd

    nc = _get_nc()
    in_maps = _prep_core_inputs(inputs, wq, bq, wk, wv, wo)
    res = run_bass_kernel_spmd(nc, in_maps, list(range(N_CORES))).results

    wo_ = np.asarray(wo, np.float32)
    bias = (
        np.asarray(bo, np.float32)
        + wo_ @ np.tile(np.asarray(bv, np.float32), NUM_HEADS)
    )
    out = np.empty((B, S, D), np.float32)
    for b in range(B):
        acc = res[b * G]["part"].astype(np.float32).copy()
        for g in range(1, G):
            acc += res[b * G + g]["part"]
        out[b] = acc + bias
    return out


# revision 10
# speedup vs baseline: 1.1551x; 1.0073x over previous
"""MQA self-attention kernel for Trainium2, 8 NeuronCores.

Reference computation (fp32):
    q = x @ wq.T + bq        -> [B,S,1024] -> heads via (hidden num_heads) split
    k = x @ wk.T + bk        -> [B,S,64]  (single shared KV head)
    v = x @ wv.T + bv
    scores = q @ k.T / 8 ; attn = softmax(scores) ; h = attn @ v
    out = merge_heads(h) @ wo.T + bo

Sharding (8 cores, no collectives): core c handles batch b=c//4 and head
group g=c%4 (4 of the 16 q-heads).  The shared K/V head is replicated.
Each core returns the partial output h_g @ wo_g.T [S, D]; the host sums
the 4 head-group partials per batch and adds the bias terms.

Per-core schedule (PE-bound at ~280k cycles):
 - Projections in fp32r (full PE rate at N=512).  K lands in psum rows
   0:64 (wvkT = [wk.T | wv.T]) so its evac needs no partition shift; the
   KT row block is duplicated to partitions 64:128 so odd heads' scores
   matmuls can run against QT stored in the other partition half.
 - scoresT[sk,sq] = KT.T @ QT per head in [128,1024] psum tiles.
 - exp is split across two engines (per key tile): ACT runs the real
   Exp activation (bf16 out); DVE computes a Schraudolph-style exp2 via
   one tensor_scalar (x*A+B -> int16, bitcast bf16).  Softmax
   renormalizes, so the ~3% sawtooth error mostly cancels; measured
   end-to-end rel err ~1e-2 vs the 2e-2 gate.
 - PV is flipped: the exp tile [sk 128, sq 128] is the stationary
   operand and V' = [V | 1] [128, 65] moves, so each matmul costs 65
   moving rows instead of 512 (PE charges by moving dim only).  The
   ones column accumulates the softmax denominator in the same psum
   tile ([sq, 65], col 64 = sum).  The 4 interleaved regions share one
   2KB zero region: start/stop bracket the whole bank's group.
 - normalize on evac: DVE reciprocal of the strided denominators, then
   DVE tensor_scalar / ACT Copy-with-scale write normalized bf16 h for
   a head pair packed [sq, h_even|h_odd].
 - DMA-transpose (free XBAR engine) flips each [sq 128, 128] pair block
   into ht [128 qd, sq] for the out projection (bf16, fp32 accum).
 - block 0's out projection is interleaved into block 1's first head so
   the PE never waits on the transpose latency.
"""

import numpy as np

NUM_HEADS = 16
Dh = 64
B, S, D = 2, 2048, 1024
G = 4            # head groups (cores per batch)
HG = 4           # heads per group
QD = HG * Dh     # 256 local q dims
NK = D // 128    # 8 contraction tiles for projections
NSK = S // 128   # 16 key tiles
W = 512          # matmul moving width
NB = 2           # sq blocks of 1024
BLK = 1024
N_CORES = 8

# Schraudolph exp2 constants (bf16 bit domain), score scale 1/8 folded in.
EXP_A = float(128.0 / np.log(2.0) * 0.125)
EXP_B = float(127.0 * 128.0 - 128.0 * np.log2(1.03279))

# exp engine split per (sk, half): 13 of 32 half-tiles go to DVE
# (schraudolph), spread evenly so ACT/DVE ping-pong without starving PE.
DVE_HALF = frozenset(i for i in range(32) if ((i + 1) * 13) % 32 < 13)
# normalize-mul engine split per sq-tile: these go to ACT Copy-with-scale.
ACT_MUL = frozenset()

_CACHE = {}


def _build_nc():
    from contextlib import ExitStack

    import concourse.bass as bass
    import concourse.mybir as mybir
    import concourse.tile as tile
    from concourse import bacc
    from concourse.masks import make_identity

    F32 = mybir.dt.float32
    F32R = mybir.dt.float32r
    BF16 = mybir.dt.bfloat16
    I16 = mybir.dt.int16
    EXP = mybir.ActivationFunctionType.Exp
    COPY = mybir.ActivationFunctionType.Copy
    MUL = mybir.AluOpType.mult
    ADD = mybir.AluOpType.add

    nc = bacc.Bacc("TRN2", target_bir_lowering=False, debug=False)

    xT = nc.declare_dram_parameter("xT", [D, S], F32R, isOutput=False)
    wqT = nc.declare_dram_parameter("wqT", [D, QD], F32R, isOutput=False)
    wvkT = nc.declare_dram_parameter("wvkT", [D, 128], F32R, isOutput=False)
    woT = nc.declare_dram_parameter("woT", [QD, D], BF16, isOutput=False)
    bqp = nc.declare_dram_parameter("bq", [128, 2], F32, isOutput=False)
    part = nc.declare_dram_parameter("part", [S, D], F32, isOutput=True)

    with tile.TileContext(nc) as tc, ExitStack() as ctx:
        const = ctx.enter_context(tc.tile_pool(name="const", bufs=1))
        persist = ctx.enter_context(tc.tile_pool(name="persist", bufs=1))

        wq_sb = const.tile([128, NK * QD], F32R)    # ktile kt at cols [kt*QD:+QD]
        wvk_sb = const.tile([128, NK * 128], F32R)  # cols 0:64 = wkT, 64:128 = wvT
        wo_sb = const.tile([128, 2 * D], BF16)      # qd-ktile p at cols [p*D:+D]
        bq_sb = const.tile([128, 2], F32)
        ident = const.tile([128, 128], F32)

        kt_sb = persist.tile([128, S], F32R)        # KT dup in both partition halves
        qt_sb = persist.tile([128, 2 * S], F32R)    # pass p: heads (2p,2p+1) stacked
        v1_sb = persist.tile([128, NSK * 65], BF16)  # V' tile sk at cols [sk*65:+65]
        ht_sb = persist.tile([128, 2 * BLK], BF16)  # block-local hT, pair p at [p*BLK:+BLK]

        make_identity(nc, ident[:])
        nc.vector.memset(v1_sb[:, 64:NSK * 65:65], 1.0)

        # ---- Phase 0/1: DMAs + projections ----------------------------
        with tc.tile_pool(name="xp", bufs=1) as xp:
            x_sb = xp.tile([128, NK * S], F32R)     # xT ktile kt at cols [kt*S:+S]
            vt_sb = xp.tile([128, S], F32)          # VT in rows 64:128

            # per-kt bundles (weights + x half-tiles) alternate SP/Pool queues
            for kt in range(NK):
                eng = nc.sync if kt % 2 == 0 else nc.gpsimd
                eng.dma_start(wvk_sb[:, kt * 128:(kt + 1) * 128], wvkT[kt * 128:(kt + 1) * 128, :])
                eng.dma_start(
                    x_sb[:, kt * S: kt * S + BLK],
                    xT[kt * 128:(kt + 1) * 128, 0:BLK],
                )
                eng.dma_start(wq_sb[:, kt * QD:(kt + 1) * QD], wqT[kt * 128:(kt + 1) * 128, :])
                eng.dma_start(
                    x_sb[:, kt * S + BLK: kt * S + 2 * BLK],
                    xT[kt * 128:(kt + 1) * 128, BLK:2 * BLK],
                )
            nc.sync.dma_start(bq_sb[:], bqp[:, :])
            for p in range(2):
                nc.sync.dma_start(wo_sb[:, p * D:(p + 1) * D], woT[p * 128:(p + 1) * 128, :])

            with (
                tc.tile_pool(name="vkps", bufs=1, space="PSUM") as vkps,
                tc.tile_pool(name="qps", bufs=1, space="PSUM") as qps,
            ):
                vk_ps = [vkps.tile([128, BLK], F32, name=f"vk{h}") for h in range(2)]
                q_ps = [qps.tile([128, BLK], F32, name=f"q{h}") for h in range(2)]
                for kt in range(NK):
                    for h in range(2):
                        nc.tensor.matmul(
                            vk_ps[h][:, 0:W],
                            lhsT=wvk_sb[:, kt * 128:(kt + 1) * 128],
                            rhs=x_sb[:, kt * S + h * BLK: kt * S + h * BLK + W],
                            start=(kt == 0), stop=(kt == NK - 1),
                        )
                        nc.tensor.matmul(
                            vk_ps[h][:, W:BLK],
                            lhsT=wvk_sb[:, kt * 128:(kt + 1) * 128],
                            rhs=x_sb[:, kt * S + h * BLK + W: kt * S + (h + 1) * BLK],
                            start=(kt == 0), stop=(kt == NK - 1),
                        )
                        for n in range(2):
                            nc.tensor.matmul(
                                q_ps[h][:, n * W:(n + 1) * W],
                                lhsT=wq_sb[:, kt * QD: kt * QD + 128],
                                rhs=x_sb[:, kt * S + h * BLK + n * W: kt * S + h * BLK + (n + 1) * W],
                                start=(kt == 0), stop=(kt == NK - 1),
                            )
                # evacs: KT + VT on ACT, Q pass0 (+bias) on DVE
                for h in range(2):
                    nc.scalar.copy(kt_sb[0:64, h * BLK:(h + 1) * BLK], vk_ps[h][0:64, :])
                    nc.scalar.copy(vt_sb[64:128, h * BLK:(h + 1) * BLK], vk_ps[h][64:128, :])
                    nc.vector.tensor_scalar_add(
                        qt_sb[:, h * BLK:(h + 1) * BLK], q_ps[h][:, :], bq_sb[:, 0:1]
                    )
                nc.gpsimd.tensor_copy(kt_sb[64:128, :], kt_sb[0:64, :])

            # Q pass 1 (heads 2,3) reuses freed banks; V' transposes after
            with (
                tc.tile_pool(name="qps2", bufs=1, space="PSUM") as qps2,
                tc.tile_pool(name="trps", bufs=2, space="PSUM") as trps,
            ):
                q_ps2 = [qps2.tile([128, BLK], F32, name=f"q2{h}") for h in range(2)]
                for kt in range(NK):
                    for h in range(2):
                        for n in range(2):
                            nc.tensor.matmul(
                                q_ps2[h][:, n * W:(n + 1) * W],
                                lhsT=wq_sb[:, kt * QD + 128: kt * QD + 256],
                                rhs=x_sb[:, kt * S + h * BLK + n * W: kt * S + h * BLK + (n + 1) * W],
                                start=(kt == 0), stop=(kt == NK - 1),
                            )
                for sk in range(NSK):
                    tr_ps = trps.tile([128, Dh], F32, name="trp")
                    nc.tensor.transpose(
                        tr_ps[:], vt_sb[64:128, sk * 128:(sk + 1) * 128],
                        ident[64:128, 64:128],
                    )
                    if sk % 2 == 0:
                        nc.scalar.copy(v1_sb[:, sk * 65: sk * 65 + 64], tr_ps[:])
                    else:
                        nc.vector.tensor_copy(v1_sb[:, sk * 65: sk * 65 + 64], tr_ps[:])
                for h in range(2):
                    nc.vector.tensor_scalar_add(
                        qt_sb[:, S + h * BLK: S + (h + 1) * BLK], q_ps2[h][:, :], bq_sb[:, 1:2]
                    )

        # ---- Phase 2: attention + output projection --------------------
        with (
            tc.tile_pool(name="expp", bufs=8) as expp,
            tc.tile_pool(name="scps", bufs=5, space="PSUM") as scps,
            tc.tile_pool(name="hps", bufs=1, space="PSUM") as hps,
            tc.tile_pool(name="outps", bufs=1, space="PSUM") as outps,
            tc.tile_pool(name="smalls", bufs=2) as smalls,
            tc.tile_pool(name="hnp", bufs=2) as hnp,
            tc.tile_pool(name="osbp", bufs=3) as osbp,
        ):
            h_ps = [hps.tile([128, W], F32, name=f"hb{i}") for i in range(2)]

            def emit_out_chunk(blk, s, tail=False):
                # one s-chunk of the block's output projection: 2 n-halves.
                # The tail (nothing left to interleave with) borrows sc-pool
                # banks so several chunks can be in flight at once.
                for n in range(2):
                    if tail:
                        o_ps = scps.tile([128, W], F32, name="sc")
                    else:
                        o_ps = outps.tile([128, W], F32, name="ops")
                    for p in range(2):
                        nc.tensor.matmul(
                            o_ps[:],
                            lhsT=ht_sb[:, p * BLK + s * 128: p * BLK + (s + 1) * 128],
                            rhs=wo_sb[:, p * D + n * W: p * D + (n + 1) * W],
                            start=(p == 0), stop=(p == 1),
                        )
                    o_sb = osbp.tile([128, W], F32, name="osb")
                    if (s + n) % 2 == 0:
                        nc.scalar.copy(o_sb[:], o_ps[:])
                    else:
                        nc.vector.tensor_copy(o_sb[:], o_ps[:])
                    deng = nc.gpsimd if (s + n) % 2 == 0 else nc.sync
                    deng.dma_start(
                        part[blk * BLK + s * 128: blk * BLK + (s + 1) * 128,
                             n * W:(n + 1) * W],
                        o_sb[:],
                    )

            for blk in range(NB):
                hn_t = None
                for hl in range(HG):
                    half, pair = hl % 2, hl // 2
                    rows = slice(0, 64) if half == 0 else slice(64, 128)
                    qcol = pair * S + blk * BLK
                    exp_tiles = [None] * NSK

                    def emit_pv(sk):
                        et0, et1 = exp_tiles[sk]
                        for t in range(8):
                            et = et0 if t < 4 else et1
                            c = t % 4
                            nc.tensor.matmul(
                                h_ps[t // 4][:, c * 65: c * 65 + 65],
                                lhsT=et[:, c * 128:(c + 1) * 128],
                                rhs=v1_sb[:, sk * 65:(sk + 1) * 65],
                                start=(sk == 0 and c == 0),
                                stop=(sk == NSK - 1 and c == 3),
                            )

                    for sk in range(NSK):
                        ets = []
                        for n in range(2):
                            sc = scps.tile([128, W], F32, name="sc")
                            nc.tensor.matmul(
                                sc[:],
                                lhsT=kt_sb[rows, sk * 128:(sk + 1) * 128],
                                rhs=qt_sb[rows, qcol + n * W: qcol + (n + 1) * W],
                                start=True, stop=True,
                            )
                            et = expp.tile([128, W], BF16, name="et")
                            if sk * 2 + n in DVE_HALF:
                                nc.vector.tensor_scalar(
                                    et[:].bitcast(I16), sc[:], EXP_A, EXP_B, MUL, ADD
                                )
                            else:
                                nc.scalar.activation(et[:], sc[:], EXP, scale=0.125)
                            ets.append(et)
                        exp_tiles[sk] = ets
                        # interleave the previous block's out projection into
                        # this block's first head so PE never waits on the
                        # ht transposes at the block boundary
                        if blk == 1 and hl == 0 and sk < 8:
                            emit_out_chunk(0, sk)
                        if sk >= 2:
                            emit_pv(sk - 2)
                    emit_pv(NSK - 2)
                    emit_pv(NSK - 1)

                    # normalize + pack head pair [sq, h_even|h_odd] in bf16
                    if half == 0:
                        hn_t = hnp.tile([128, 8 * 128], BF16, name="hn")
                    rec = smalls.tile([128, 8], F32, name="rec")
                    for i in range(2):
                        nc.vector.reciprocal(
                            rec[:, i * 4:(i + 1) * 4], h_ps[i][:, 64:260:65]
                        )
                    for t in range(8):
                        i, c = t // 4, t % 4
                        dst = hn_t[:, t * 128 + half * 64: t * 128 + half * 64 + 64]
                        if t in ACT_MUL:
                            nc.scalar.activation(
                                dst, h_ps[i][:, c * 65: c * 65 + 64],
                                COPY, scale=rec[:, t:t + 1],
                            )
                        else:
                            nc.vector.tensor_scalar(
                                dst, h_ps[i][:, c * 65: c * 65 + 64],
                                rec[:, t:t + 1], None, MUL,
                            )
                    if half == 1:
                        for t in range(8):
                            nc.sync.dma_start_transpose(
                                ht_sb[:, pair * BLK + t * 128: pair * BLK + (t + 1) * 128],
                                hn_t[:, t * 128:(t + 1) * 128],
                            )

                if blk == 1:
                    for s in range(8):
                        emit_out_chunk(1, s, tail=True)
                # block 0's out projection is emitted inside block 1 head 0

    nc.finalize()
    return nc


def _get_nc():
    if "nc" not in _CACHE:
        _CACHE["nc"] = _build_nc()
    return _CACHE["nc"]


def _to_bf16_bits(a):
    """fp32 -> bf16 bits (round-to-nearest-even), as uint16."""
    u = np.asarray(a, np.float32).view(np.uint32)
    return ((u + 0x7FFF + ((u >> 16) & 1)) >> 16).astype(np.uint16)


def _prep_core_inputs(inputs, wq, bq, wk, wv, wo):
    """Host-side shard prep: per-core transposed/rearranged operands."""
    xT = [np.ascontiguousarray(np.asarray(inputs[b], np.float32).T) for b in range(B)]
    wq3 = np.asarray(wq, np.float32).reshape(Dh, NUM_HEADS, D)
    bq2 = np.asarray(bq, np.float32).reshape(Dh, NUM_HEADS)
    wvkT = np.ascontiguousarray(
        np.concatenate([np.asarray(wk, np.float32).T, np.asarray(wv, np.float32).T], axis=1)
    )  # [1024, 128], K first
    wo_ = np.asarray(wo, np.float32)

    in_maps = []
    for c in range(N_CORES):
        b, g = divmod(c, G)
        heads = [g * HG + hl for hl in range(HG)]
        wqT_g = np.ascontiguousarray(
            np.concatenate([wq3[:, h, :].T for h in heads], axis=1)
        )  # [1024, 256]
        bq_g = np.ascontiguousarray(
            np.concatenate([bq2[:, h] for h in heads]).reshape(2, 128).T
        )  # [128, 2]: col p = heads (2p, 2p+1) stacked
        woT_g = _to_bf16_bits(wo_[:, g * QD:(g + 1) * QD].T)  # [256, 1024] bf16
        in_maps.append({
            "xT": xT[b],
            "wqT": wqT_g,
            "wvkT": wvkT,
            "woT": woT_g,
            "bq": bq_g,
        })
    return in_maps


def kernel(inputs, wq, bq, wk, bk, wv, bv, wo, bo):
    from concourse.bass_utils import run_bass_kernel_spmd

    nc = _get_nc()
    in_maps = _prep_core_inputs(inputs, wq, bq, wk, wv, wo)
    res = run_bass_kernel_spmd(nc, in_maps, list(range(N_CORES))).results

    wo_ = np.asarray(wo, np.float32)
    bias = (
        np.asarray(bo, np.float32)
        + wo_ @ np.tile(np.asarray(bv, np.float32), NUM_HEADS)
    )
    out = np.empty((B, S, D), np.float32)
    for b in range(B):
        acc = res[b * G]["part"].astype(np.float32).copy()
        for g in range(1, G):
            acc += res[b * G + g]["part"]
        out[b] = acc + bias
    return out


# revision 12
# speedup vs baseline: 1.1767x; 1.0187x over previous
"""MQA self-attention kernel for Trainium2, 8 NeuronCores.

Reference computation (fp32):
    q = x @ wq.T + bq        -> [B,S,1024] -> heads via (hidden num_heads) split
    k = x @ wk.T + bk        -> [B,S,64]  (single shared KV head)
    v = x @ wv.T + bv
    scores = q @ k.T / 8 ; attn = softmax(scores) ; h = attn @ v
    out = merge_heads(h) @ wo.T + bo

Sharding (8 cores, no collectives): core c handles batch b=c//4 and head
group g=c%4 (4 of the 16 q-heads).  The shared K/V head is replicated.
Each core returns the partial output h_g @ wo_g.T [S, D]; the host sums
the 4 head-group partials per batch and adds the bias terms.

Per-core schedule (PE-bound at ~280k cycles):
 - Projections in fp32r (full PE rate at N=512).  K lands in psum rows
   0:64 (wvkT = [wk.T | wv.T]) so its evac needs no partition shift; the
   KT row block is duplicated to partitions 64:128 so odd heads' scores
   matmuls can run against QT stored in the other partition half.
 - scoresT[sk,sq] = KT.T @ QT per head in [128,1024] psum tiles.
 - exp is split across two engines (per key tile): ACT runs the real
   Exp activation (bf16 out); DVE computes a Schraudolph-style exp2 via
   one tensor_scalar (x*A+B -> int16, bitcast bf16).  Softmax
   renormalizes, so the ~3% sawtooth error mostly cancels; measured
   end-to-end rel err ~1e-2 vs the 2e-2 gate.
 - PV is flipped: the exp tile [sk 128, sq 128] is the stationary
   operand and V' = [V | 1] [128, 65] moves, so each matmul costs 65
   moving rows instead of 512 (PE charges by moving dim only).  The
   ones column accumulates the softmax denominator in the same psum
   tile ([sq, 65], col 64 = sum).  The 4 interleaved regions share one
   2KB zero region: start/stop bracket the whole bank's group.
 - normalize on evac: DVE reciprocal of the strided denominators, then
   DVE tensor_scalar / ACT Copy-with-scale write normalized bf16 h for
   a head pair packed [sq, h_even|h_odd].
 - DMA-transpose (free XBAR engine) flips each [sq 128, 128] pair block
   into ht [128 qd, sq] for the out projection (bf16, fp32 accum).
 - block 0's out projection is interleaved into block 1's first head so
   the PE never waits on the transpose latency.
"""

import numpy as np

NUM_HEADS = 16
Dh = 64
B, S, D = 2, 2048, 1024
G = 4            # head groups (cores per batch)
HG = 4           # heads per group
QD = HG * Dh     # 256 local q dims
NK = D // 128    # 8 contraction tiles for projections
NSK = S // 128   # 16 key tiles
W = 512          # matmul moving width
NB = 2           # sq blocks of 1024
BLK = 1024
N_CORES = 8

# Schraudolph exp2 constants (bf16 bit domain), score scale 1/8 folded in.
EXP_A = float(128.0 / np.log(2.0) * 0.125)
EXP_B = float(127.0 * 128.0 - 128.0 * np.log2(1.03279))

# exp engine split per (sk, half): 13 of 32 half-tiles go to DVE
# (schraudolph), spread evenly so ACT/DVE ping-pong without starving PE.
DVE_HALF = frozenset(i for i in range(32) if ((i + 1) * 13) % 32 < 13)
# the final head feeds the output-projection tail: shift two more halves to
# DVE there so ACT's backlog doesn't delay the tail's psum evacs.
DVE_HALF_LAST = DVE_HALF | {1, 17}

_CACHE = {}


def _build_nc():
    from contextlib import ExitStack

    import concourse.bass as bass
    import concourse.mybir as mybir
    import concourse.tile as tile
    from concourse import bacc
    from concourse.masks import make_identity

    F32 = mybir.dt.float32
    F32R = mybir.dt.float32r
    BF16 = mybir.dt.bfloat16
    I16 = mybir.dt.int16
    EXP = mybir.ActivationFunctionType.Exp
    COPY = mybir.ActivationFunctionType.Copy
    MUL = mybir.AluOpType.mult
    ADD = mybir.AluOpType.add

    nc = bacc.Bacc("TRN2", target_bir_lowering=False, debug=False)

    xT = nc.declare_dram_parameter("xT", [D, S], F32R, isOutput=False)
    wqT = nc.declare_dram_parameter("wqT", [D, QD], F32R, isOutput=False)
    wvkT = nc.declare_dram_parameter("wvkT", [D, 128], F32R, isOutput=False)
    woT = nc.declare_dram_parameter("woT", [QD, D], BF16, isOutput=False)
    bqp = nc.declare_dram_parameter("bq", [128, 2], F32, isOutput=False)
    part = nc.declare_dram_parameter("part", [S, D], F32, isOutput=True)

    with tile.TileContext(nc) as tc, ExitStack() as ctx:
        const = ctx.enter_context(tc.tile_pool(name="const", bufs=1))
        persist = ctx.enter_context(tc.tile_pool(name="persist", bufs=1))

        wq_sb = const.tile([128, NK * QD], F32R)    # ktile kt at cols [kt*QD:+QD]
        wvk_sb = const.tile([128, NK * 128], F32R)  # cols 0:64 = wkT, 64:128 = wvT
        wo_sb = const.tile([128, 2 * D], BF16)      # qd-ktile p at cols [p*D:+D]
        bq_sb = const.tile([128, 2], F32)
        ident = const.tile([128, 128], F32)

        kt_sb = persist.tile([128, S], F32R)        # KT dup in both partition halves
        qt_sb = persist.tile([128, 2 * S], F32R)    # pass p: heads (2p,2p+1) stacked
        v1_sb = persist.tile([128, NSK * 65], BF16)  # V' tile sk at cols [sk*65:+65]
        ht_sb = persist.tile([128, 2 * BLK], BF16)  # block-local hT, pair p at [p*BLK:+BLK]

        make_identity(nc, ident[:])
        nc.vector.memset(v1_sb[:, 64:NSK * 65:65], 1.0)

        # ---- Phase 0/1: DMAs + projections ----------------------------
        with tc.tile_pool(name="xp", bufs=1) as xp:
            x_sb = xp.tile([128, NK * S], F32R)     # xT ktile kt at cols [kt*S:+S]
            vt_sb = xp.tile([128, S], F32)          # VT in rows 64:128

            # per-kt bundles (weights + x half-tiles) alternate SP/Pool queues
            for kt in range(NK):
                eng = nc.sync if kt % 2 == 0 else nc.gpsimd
                eng.dma_start(wvk_sb[:, kt * 128:(kt + 1) * 128], wvkT[kt * 128:(kt + 1) * 128, :])
                eng.dma_start(
                    x_sb[:, kt * S: kt * S + BLK],
                    xT[kt * 128:(kt + 1) * 128, 0:BLK],
                )
                eng.dma_start(wq_sb[:, kt * QD:(kt + 1) * QD], wqT[kt * 128:(kt + 1) * 128, :])
                eng.dma_start(
                    x_sb[:, kt * S + BLK: kt * S + 2 * BLK],
                    xT[kt * 128:(kt + 1) * 128, BLK:2 * BLK],
                )
            nc.sync.dma_start(bq_sb[:], bqp[:, :])
            for p in range(2):
                nc.sync.dma_start(wo_sb[:, p * D:(p + 1) * D], woT[p * 128:(p + 1) * 128, :])

            with (
                tc.tile_pool(name="vkps", bufs=1, space="PSUM") as vkps,
                tc.tile_pool(name="qps", bufs=1, space="PSUM") as qps,
            ):
                vk_ps = [vkps.tile([128, BLK], F32, name=f"vk{h}") for h in range(2)]
                q_ps = [qps.tile([128, BLK], F32, name=f"q{h}") for h in range(2)]
                for kt in range(NK):
                    for h in range(2):
                        nc.tensor.matmul(
                            vk_ps[h][:, 0:W],
                            lhsT=wvk_sb[:, kt * 128:(kt + 1) * 128],
                            rhs=x_sb[:, kt * S + h * BLK: kt * S + h * BLK + W],
                            start=(kt == 0), stop=(kt == NK - 1),
                        )
                        nc.tensor.matmul(
                            vk_ps[h][:, W:BLK],
                            lhsT=wvk_sb[:, kt * 128:(kt + 1) * 128],
                            rhs=x_sb[:, kt * S + h * BLK + W: kt * S + (h + 1) * BLK],
                            start=(kt == 0), stop=(kt == NK - 1),
                        )
                        for n in range(2):
                            nc.tensor.matmul(
                                q_ps[h][:, n * W:(n + 1) * W],
                                lhsT=wq_sb[:, kt * QD: kt * QD + 128],
                                rhs=x_sb[:, kt * S + h * BLK + n * W: kt * S + h * BLK + (n + 1) * W],
                                start=(kt == 0), stop=(kt == NK - 1),
                            )
                # evacs: fine-grained 512-col chunks so the first
                # scores matmuls of phase 2 unblock as early as possible
                for h in range(2):
                    for n in range(2):
                        c0 = h * BLK + n * W
                        nc.scalar.copy(kt_sb[0:64, c0:c0 + W], vk_ps[h][0:64, n * W:(n + 1) * W])
                        nc.gpsimd.tensor_copy(kt_sb[64:128, c0:c0 + W], kt_sb[0:64, c0:c0 + W])
                        nc.vector.tensor_scalar_add(
                            qt_sb[:, c0:c0 + W], q_ps[h][:, n * W:(n + 1) * W], bq_sb[:, 0:1]
                        )
                    nc.scalar.copy(vt_sb[64:128, h * BLK:(h + 1) * BLK], vk_ps[h][64:128, :])

            # Q pass 1 (heads 2,3) reuses freed banks; V' transposes after
            with (
                tc.tile_pool(name="qps2", bufs=1, space="PSUM") as qps2,
                tc.tile_pool(name="trps", bufs=2, space="PSUM") as trps,
            ):
                q_ps2 = [qps2.tile([128, BLK], F32, name=f"q2{h}") for h in range(2)]

                def q1_mms(h):
                    for kt in range(NK):
                        for n in range(2):
                            nc.tensor.matmul(
                                q_ps2[h][:, n * W:(n + 1) * W],
                                lhsT=wq_sb[:, kt * QD + 128: kt * QD + 256],
                                rhs=x_sb[:, kt * S + h * BLK + n * W: kt * S + h * BLK + (n + 1) * W],
                                start=(kt == 0), stop=(kt == NK - 1),
                            )

                def v1_transposes(lo, hi):
                    for sk in range(lo, hi):
                        tr_ps = trps.tile([128, Dh], F32, name="trp")
                        nc.tensor.transpose(
                            tr_ps[:], vt_sb[64:128, sk * 128:(sk + 1) * 128],
                            ident[64:128, 64:128],
                        )
                        if sk % 2 == 0:
                            nc.scalar.copy(v1_sb[:, sk * 65: sk * 65 + 64], tr_ps[:])
                        else:
                            nc.vector.tensor_copy(v1_sb[:, sk * 65: sk * 65 + 64], tr_ps[:])

                q1_mms(0)
                v1_transposes(0, NSK // 2)
                q1_mms(1)
                v1_transposes(NSK // 2, NSK)
                for h in range(2):
                    for n in range(2):
                        nc.vector.tensor_scalar_add(
                            qt_sb[:, S + h * BLK + n * W: S + h * BLK + (n + 1) * W],
                            q_ps2[h][:, n * W:(n + 1) * W], bq_sb[:, 1:2]
                        )

        # ---- Phase 2: attention + output projection --------------------
        with (
            tc.tile_pool(name="expp", bufs=8) as expp,
            tc.tile_pool(name="scps", bufs=5, space="PSUM") as scps,
            tc.tile_pool(name="hps", bufs=1, space="PSUM") as hps,
            tc.tile_pool(name="outps", bufs=1, space="PSUM") as outps,
            tc.tile_pool(name="smalls", bufs=2) as smalls,
            tc.tile_pool(name="hnp", bufs=2) as hnp,
            tc.tile_pool(name="osbp", bufs=3) as osbp,
        ):
            h_ps = [hps.tile([128, W], F32, name=f"hb{i}") for i in range(2)]

            def emit_out_chunk(blk, s, tail=False):
                # one s-chunk of the block's output projection: 2 n-halves.
                # The tail (nothing left to interleave with) borrows sc-pool
                # banks so several chunks can be in flight at once.
                for n in range(2):
                    if tail:
                        o_ps = scps.tile([128, W], F32, name="sc")
                    else:
                        o_ps = outps.tile([128, W], F32, name="ops")
                    for p in range(2):
                        nc.tensor.matmul(
                            o_ps[:],
                            lhsT=ht_sb[:, p * BLK + s * 128: p * BLK + (s + 1) * 128],
                            rhs=wo_sb[:, p * D + n * W: p * D + (n + 1) * W],
                            start=(p == 0), stop=(p == 1),
                        )
                    o_sb = osbp.tile([128, W], F32, name="osb")
                    if (s + n) % 2 == 0:
                        nc.scalar.copy(o_sb[:], o_ps[:])
                    else:
                        nc.vector.tensor_copy(o_sb[:], o_ps[:])
                    deng = nc.gpsimd if (s + n) % 2 == 0 else nc.sync
                    deng.dma_start(
                        part[blk * BLK + s * 128: blk * BLK + (s + 1) * 128,
                             n * W:(n + 1) * W],
                        o_sb[:],
                    )

            for blk in range(NB):
                hn_t = None
                for hl in range(HG):
                    half, pair = hl % 2, hl // 2
                    rows = slice(0, 64) if half == 0 else slice(64, 128)
                    qcol = pair * S + blk * BLK
                    exp_tiles = [None] * NSK

                    def emit_pv(sk):
                        et0, et1 = exp_tiles[sk]
                        for t in range(8):
                            et = et0 if t < 4 else et1
                            c = t % 4
                            nc.tensor.matmul(
                                h_ps[t // 4][:, c * 65: c * 65 + 65],
                                lhsT=et[:, c * 128:(c + 1) * 128],
                                rhs=v1_sb[:, sk * 65:(sk + 1) * 65],
                                start=(sk == 0 and c == 0),
                                stop=(sk == NSK - 1 and c == 3),
                            )

                    for sk in range(NSK):
                        ets = []
                        for n in range(2):
                            sc = scps.tile([128, W], F32, name="sc")
                            nc.tensor.matmul(
                                sc[:],
                                lhsT=kt_sb[rows, sk * 128:(sk + 1) * 128],
                                rhs=qt_sb[rows, qcol + n * W: qcol + (n + 1) * W],
                                start=True, stop=True,
                            )
                            et = expp.tile([128, W], BF16, name="et")
                            dset = DVE_HALF_LAST if (blk == 1 and hl == 3) else DVE_HALF
                            if sk * 2 + n in dset:
                                nc.vector.tensor_scalar(
                                    et[:].bitcast(I16), sc[:], EXP_A, EXP_B, MUL, ADD
                                )
                            else:
                                nc.scalar.activation(et[:], sc[:], EXP, scale=0.125)
                            ets.append(et)
                        exp_tiles[sk] = ets
                        # interleave the previous block's out projection into
                        # this block's first head so PE never waits on the
                        # ht transposes at the block boundary
                        if blk == 1 and hl == 0 and sk < 8:
                            emit_out_chunk(0, sk)
                        if sk >= 2:
                            emit_pv(sk - 2)
                    emit_pv(NSK - 2)
                    emit_pv(NSK - 1)

                    # normalize + pack head pair [sq, h_even|h_odd] in bf16
                    if half == 0:
                        hn_t = hnp.tile([128, 8 * 128], BF16, name="hn")
                    rec = smalls.tile([128, 8], F32, name="rec")
                    for i in range(2):
                        nc.vector.reciprocal(
                            rec[:, i * 4:(i + 1) * 4], h_ps[i][:, 64:260:65]
                        )
                    for i in range(2):
                        h3 = h_ps[i][:, 0:260].rearrange(
                            "p (t j) -> p t j", t=4)[:, :, 0:64]
                        o3 = hn_t[:, i * W:(i + 1) * W].rearrange(
                            "p (t j) -> p t j", t=4)[:, :, half * 64: half * 64 + 64]
                        r3 = rec[:, i * 4:(i + 1) * 4].unsqueeze(2).broadcast_to([128, 4, 64])
                        nc.vector.tensor_tensor(out=o3, in0=h3, in1=r3, op=MUL)
                    if half == 1:
                        for t in range(8):
                            nc.sync.dma_start_transpose(
                                ht_sb[:, pair * BLK + t * 128: pair * BLK + (t + 1) * 128],
                                hn_t[:, t * 128:(t + 1) * 128],
                            )

                if blk == 1:
                    for s in range(8):
                        emit_out_chunk(1, s, tail=True)
                # block 0's out projection is emitted inside block 1 head 0

    nc.finalize()
    return nc


def _get_nc():
    if "nc" not in _CACHE:
        _CACHE["nc"] = _build_nc()
    return _CACHE["nc"]


def _to_bf16_bits(a):
    """fp32 -> bf16 bits (round-to-nearest-even), as uint16."""
    u = np.asarray(a, np.float32).view(np.uint32)
    return ((u + 0x7FFF + ((u >> 16) & 1)) >> 16).astype(np.uint16)


def _prep_core_inputs(inputs, wq, bq, wk, wv, wo):
    """Host-side shard prep: per-core transposed/rearranged operands."""
    xT = [np.ascontiguousarray(np.asarray(inputs[b], np.float32).T) for b in range(B)]
    wq3 = np.asarray(wq, np.float32).reshape(Dh, NUM_HEADS, D)
    bq2 = np.asarray(bq, np.float32).reshape(Dh, NUM_HEADS)
    wvkT = np.ascontiguousarray(
        np.concatenate([np.asarray(wk, np.float32).T, np.asarray(wv, np.float32).T], axis=1)
    )  # [1024, 128], K first
    wo_ = np.asarray(wo, np.float32)

    in_maps = []
    for c in range(N_CORES):
        b, g = divmod(c, G)
        heads = [g * HG + hl for hl in range(HG)]
        wqT_g = np.ascontiguousarray(
            np.concatenate([wq3[:, h, :].T for h in heads], axis=1)
        )  # [1024, 256]
        bq_g = np.ascontiguousarray(
            np.concatenate([bq2[:, h] for h in heads]).reshape(2, 128).T
        )  # [128, 2]: col p = heads (2p, 2p+1) stacked
        woT_g = _to_bf16_bits(wo_[:, g * QD:(g + 1) * QD].T)  # [256, 1024] bf16
        in_maps.append({
            "xT": xT[b],
            "wqT": wqT_g,
            "wvkT": wvkT,
            "woT": woT_g,
            "bq": bq_g,
        })
    return in_maps


def kernel(inputs, wq, bq, wk, bk, wv, bv, wo, bo):
    from concourse.bass_utils import run_bass_kernel_spmd

    nc = _get_nc()
    in_maps = _prep_core_inputs(inputs, wq, bq, wk, wv, wo)
    res = run_bass_kernel_spmd(nc, in_maps, list(range(N_CORES))).results

    wo_ = np.asarray(wo, np.float32)
    bias = (
        np.asarray(bo, np.float32)
        + wo_ @ np.tile(np.asarray(bv, np.float32), NUM_HEADS)
    )
    out = np.empty((B, S, D), np.float32)
    for b in range(B):
        acc = res[b * G]["part"].astype(np.float32).copy()
        for g in range(1, G):
            acc += res[b * G + g]["part"]
        out[b] = acc + bias
    return out


# revision 13
# speedup vs baseline: 1.2539x; 1.0656x over previous
"""MQA self-attention kernel for Trainium2, 8 NeuronCores.

Reference computation (fp32):
    q = x @ wq.T + bq        -> [B,S,1024] -> heads via (hidden num_heads) split
    k = x @ wk.T + bk        -> [B,S,64]  (single shared KV head)
    v = x @ wv.T + bv
    scores = q @ k.T / 8 ; attn = softmax(scores) ; h = attn @ v
    out = merge_heads(h) @ wo.T + bo

Sharding (8 cores, no collectives): core c handles batch b=c//4 and head
group g=c%4 (4 of the 16 q-heads).  The shared K/V head is replicated.
Each core returns the partial output h_g @ wo_g.T [S, D]; the host sums
the 4 head-group partials per batch and adds the bias terms.

Per-core schedule (PE-bound at ~280k cycles):
 - Projections in fp32r (full PE rate at N=512).  K lands in psum rows
   0:64 (wvkT = [wk.T | wv.T]) so its evac needs no partition shift; the
   KT row block is duplicated to partitions 64:128 so odd heads' scores
   matmuls can run against QT stored in the other partition half.
 - scoresT[sk,sq] = KT.T @ QT per head in [128,1024] psum tiles.
 - exp is split across two engines (per key tile): ACT runs the real
   Exp activation (bf16 out); DVE computes a Schraudolph-style exp2 via
   one tensor_scalar (x*A+B -> int16, bitcast bf16).  Softmax
   renormalizes, so the ~3% sawtooth error mostly cancels; measured
   end-to-end rel err ~1e-2 vs the 2e-2 gate.
 - PV is flipped: the exp tile [sk 128, sq 128] is the stationary
   operand and V' = [V | 1] [128, 65] moves, so each matmul costs 65
   moving rows instead of 512 (PE charges by moving dim only).  The
   ones column accumulates the softmax denominator in the same psum
   tile ([sq, 65], col 64 = sum).  The 4 interleaved regions share one
   2KB zero region: start/stop bracket the whole bank's group.
 - normalize on evac: DVE reciprocal of the strided denominators, then
   DVE tensor_scalar / ACT Copy-with-scale write normalized bf16 h for
   a head pair packed [sq, h_even|h_odd].
 - DMA-transpose (free XBAR engine) flips each [sq 128, 128] pair block
   into ht [128 qd, sq] for the out projection (bf16, fp32 accum).
 - block 0's out projection is interleaved into block 1's first head so
   the PE never waits on the transpose latency.
"""

import numpy as np

NUM_HEADS = 16
Dh = 64
B, S, D = 2, 2048, 1024
G = 4            # head groups (cores per batch)
HG = 4           # heads per group
QD = HG * Dh     # 256 local q dims
NK = D // 128    # 8 contraction tiles for projections
NSK = S // 128   # 16 key tiles
W = 512          # matmul moving width
NB = 2           # sq blocks of 1024
BLK = 1024
N_CORES = 8

# Schraudolph exp2 constants (bf16 bit domain), score scale 1/8 folded in.
EXP_A = float(128.0 / np.log(2.0) * 0.125)
EXP_B = float(127.0 * 128.0 - 128.0 * np.log2(1.03279))

# exp engine split per (sk, half): 13 of 32 half-tiles go to DVE
# (schraudolph), spread evenly so ACT/DVE ping-pong without starving PE.
DVE_HALF = frozenset(i for i in range(32) if ((i + 1) * 13) % 32 < 13)
# the final head feeds the output-projection tail: shift two more halves to
# DVE there so ACT's backlog doesn't delay the tail's psum evacs.
DVE_HALF_LAST = DVE_HALF | {1, 17}

_CACHE = {}


def _build_nc():
    from contextlib import ExitStack

    import concourse.bass as bass
    import concourse.mybir as mybir
    import concourse.tile as tile
    from concourse import bacc
    from concourse.masks import make_identity

    F32 = mybir.dt.float32
    F32R = mybir.dt.float32r
    BF16 = mybir.dt.bfloat16
    I16 = mybir.dt.int16
    EXP = mybir.ActivationFunctionType.Exp
    COPY = mybir.ActivationFunctionType.Copy
    MUL = mybir.AluOpType.mult
    ADD = mybir.AluOpType.add

    nc = bacc.Bacc("TRN2", target_bir_lowering=False, debug=False)

    xT = nc.declare_dram_parameter("xT", [D, S], F32R, isOutput=False)
    wqT = nc.declare_dram_parameter("wqT", [D, QD], F32R, isOutput=False)
    wvkT = nc.declare_dram_parameter("wvkT", [D, 128], F32R, isOutput=False)
    woT = nc.declare_dram_parameter("woT", [QD, D], BF16, isOutput=False)
    bqp = nc.declare_dram_parameter("bq", [128, 2], F32, isOutput=False)
    part = nc.declare_dram_parameter("part", [S, D], F32, isOutput=True)

    with tile.TileContext(nc) as tc, ExitStack() as ctx:
        const = ctx.enter_context(tc.tile_pool(name="const", bufs=1))
        persist = ctx.enter_context(tc.tile_pool(name="persist", bufs=1))

        wq_sb = const.tile([128, NK * QD], F32R)    # ktile kt at cols [kt*QD:+QD]
        wvk_sb = const.tile([128, NK * 128], F32R)  # cols 0:64 = wkT, 64:128 = wvT
        wo_sb = const.tile([128, 2 * D], BF16)      # qd-ktile p at cols [p*D:+D]
        bq_sb = const.tile([128, 2], F32)
        ident = const.tile([128, 128], F32)

        kt_sb = persist.tile([128, S], F32R)        # KT dup in both partition halves
        qt_sb = persist.tile([128, 2 * S], F32R)    # pass p: heads (2p,2p+1) stacked
        v1_sb = persist.tile([128, NSK * 65], BF16)  # V' tile sk at cols [sk*65:+65]
        ht_sb = persist.tile([128, 2 * BLK], BF16)  # block-local hT, pair p at [p*BLK:+BLK]

        make_identity(nc, ident[:])
        nc.vector.memset(v1_sb[:, 64:NSK * 65:65], 1.0)

        # ---- Phase 0/1: DMAs + projections ----------------------------
        with tc.tile_pool(name="xp", bufs=1) as xp:
            x_sb = xp.tile([128, NK * S], F32R)     # xT ktile kt at cols [kt*S:+S]
            vt_sb = xp.tile([128, S], F32)          # VT in rows 64:128

            # per-kt bundles (weights + x half-tiles) alternate SP/Pool queues
            for kt in range(NK):
                eng = nc.sync if kt % 2 == 0 else nc.gpsimd
                eng.dma_start(wvk_sb[:, kt * 128:(kt + 1) * 128], wvkT[kt * 128:(kt + 1) * 128, :])
                eng.dma_start(
                    x_sb[:, kt * S: kt * S + BLK],
                    xT[kt * 128:(kt + 1) * 128, 0:BLK],
                )
                eng.dma_start(wq_sb[:, kt * QD:(kt + 1) * QD], wqT[kt * 128:(kt + 1) * 128, :])
                eng.dma_start(
                    x_sb[:, kt * S + BLK: kt * S + 2 * BLK],
                    xT[kt * 128:(kt + 1) * 128, BLK:2 * BLK],
                )
            nc.sync.dma_start(bq_sb[:], bqp[:, :])
            for p in range(2):
                nc.sync.dma_start(wo_sb[:, p * D:(p + 1) * D], woT[p * 128:(p + 1) * 128, :])

            with (
                tc.tile_pool(name="vkps", bufs=1, space="PSUM") as vkps,
                tc.tile_pool(name="qps", bufs=1, space="PSUM") as qps,
            ):
                vk_ps = [vkps.tile([128, BLK], F32, name=f"vk{h}") for h in range(2)]
                q_ps = [qps.tile([128, BLK], F32, name=f"q{h}") for h in range(2)]
                for kt in range(NK):
                    for h in range(2):
                        nc.tensor.matmul(
                            vk_ps[h][:, 0:W],
                            lhsT=wvk_sb[:, kt * 128:(kt + 1) * 128],
                            rhs=x_sb[:, kt * S + h * BLK: kt * S + h * BLK + W],
                            start=(kt == 0), stop=(kt == NK - 1),
                        )
                        nc.tensor.matmul(
                            vk_ps[h][:, W:BLK],
                            lhsT=wvk_sb[:, kt * 128:(kt + 1) * 128],
                            rhs=x_sb[:, kt * S + h * BLK + W: kt * S + (h + 1) * BLK],
                            start=(kt == 0), stop=(kt == NK - 1),
                        )
                        for n in range(2):
                            nc.tensor.matmul(
                                q_ps[h][:, n * W:(n + 1) * W],
                                lhsT=wq_sb[:, kt * QD: kt * QD + 128],
                                rhs=x_sb[:, kt * S + h * BLK + n * W: kt * S + h * BLK + (n + 1) * W],
                                start=(kt == 0), stop=(kt == NK - 1),
                            )
                # evacs: fine-grained 512-col chunks so the first
                # scores matmuls of phase 2 unblock as early as possible
                for h in range(2):
                    for n in range(2):
                        c0 = h * BLK + n * W
                        nc.scalar.copy(kt_sb[0:64, c0:c0 + W], vk_ps[h][0:64, n * W:(n + 1) * W])
                        nc.gpsimd.tensor_copy(kt_sb[64:128, c0:c0 + W], kt_sb[0:64, c0:c0 + W])
                        nc.vector.tensor_scalar_add(
                            qt_sb[:, c0:c0 + W], q_ps[h][:, n * W:(n + 1) * W], bq_sb[:, 0:1]
                        )
                    nc.scalar.copy(vt_sb[64:128, h * BLK:(h + 1) * BLK], vk_ps[h][64:128, :])

            # Q pass 1 (heads 2,3) reuses freed banks; V' transposes after
            with (
                tc.tile_pool(name="qps2", bufs=1, space="PSUM") as qps2,
                tc.tile_pool(name="trps", bufs=2, space="PSUM") as trps,
            ):
                q_ps2 = [qps2.tile([128, BLK], F32, name=f"q2{h}") for h in range(2)]

                def q1_mms(h):
                    for kt in range(NK):
                        for n in range(2):
                            nc.tensor.matmul(
                                q_ps2[h][:, n * W:(n + 1) * W],
                                lhsT=wq_sb[:, kt * QD + 128: kt * QD + 256],
                                rhs=x_sb[:, kt * S + h * BLK + n * W: kt * S + h * BLK + (n + 1) * W],
                                start=(kt == 0), stop=(kt == NK - 1),
                            )

                def v1_transposes(lo, hi):
                    for sk in range(lo, hi):
                        tr_ps = trps.tile([128, Dh], F32, name="trp")
                        nc.tensor.transpose(
                            tr_ps[:], vt_sb[64:128, sk * 128:(sk + 1) * 128],
                            ident[64:128, 64:128],
                        )
                        if sk % 2 == 0:
                            nc.scalar.copy(v1_sb[:, sk * 65: sk * 65 + 64], tr_ps[:])
                        else:
                            nc.vector.tensor_copy(v1_sb[:, sk * 65: sk * 65 + 64], tr_ps[:])

                q1_mms(0)
                v1_transposes(0, NSK // 2)
                q1_mms(1)
                v1_transposes(NSK // 2, NSK)
                for h in range(2):
                    for n in range(2):
                        nc.vector.tensor_scalar_add(
                            qt_sb[:, S + h * BLK + n * W: S + h * BLK + (n + 1) * W],
                            q_ps2[h][:, n * W:(n + 1) * W], bq_sb[:, 1:2]
                        )

        # ---- Phase 2: attention + output projection --------------------
        with (
            tc.tile_pool(name="expp", bufs=8) as expp,
            tc.tile_pool(name="scps", bufs=5, space="PSUM") as scps,
            tc.tile_pool(name="hps", bufs=1, space="PSUM") as hps,
            tc.tile_pool(name="outps", bufs=1, space="PSUM") as outps,
            tc.tile_pool(name="smalls", bufs=2) as smalls,
            tc.tile_pool(name="hnp", bufs=2) as hnp,
            tc.tile_pool(name="osbp", bufs=6) as osbp,
        ):
            h_ps = [hps.tile([128, W], F32, name=f"hb{i}") for i in range(2)]

            def emit_out_chunk(blk, s, tail=False):
                # one s-chunk of the block's output projection: 2 n-halves.
                # The tail (nothing left to interleave with) borrows sc-pool
                # banks so several chunks can be in flight at once.
                for n in range(2):
                    if tail:
                        o_ps = scps.tile([128, W], F32, name="sc")
                    else:
                        o_ps = outps.tile([128, W], F32, name="ops")
                    for p in range(2):
                        nc.tensor.matmul(
                            o_ps[:],
                            lhsT=ht_sb[:, p * BLK + s * 128: p * BLK + (s + 1) * 128],
                            rhs=wo_sb[:, p * D + n * W: p * D + (n + 1) * W],
                            start=(p == 0), stop=(p == 1),
                        )
                    o_sb = osbp.tile([128, W], F32, name="osb")
                    if n == 0:
                        nc.scalar.copy(o_sb[:], o_ps[:])
                    else:
                        nc.vector.tensor_copy(o_sb[:], o_ps[:])
                    deng = nc.gpsimd if n == 0 else nc.sync
                    deng.dma_start(
                        part[blk * BLK + s * 128: blk * BLK + (s + 1) * 128,
                             n * W:(n + 1) * W],
                        o_sb[:],
                    )

            for blk in range(NB):
                hn_t = None
                for hl in range(HG):
                    half, pair = hl % 2, hl // 2
                    rows = slice(0, 64) if half == 0 else slice(64, 128)
                    qcol = pair * S + blk * BLK
                    exp_tiles = [None] * NSK

                    def emit_pv(sk):
                        et0, et1 = exp_tiles[sk]
                        for t in range(8):
                            et = et0 if t < 4 else et1
                            c = t % 4
                            nc.tensor.matmul(
                                h_ps[t // 4][:, c * 65: c * 65 + 65],
                                lhsT=et[:, c * 128:(c + 1) * 128],
                                rhs=v1_sb[:, sk * 65:(sk + 1) * 65],
                                start=(sk == 0 and c == 0),
                                stop=(sk == NSK - 1 and c == 3),
                            )

                    for sk in range(NSK):
                        ets = []
                        for n in range(2):
                            sc = scps.tile([128, W], F32, name="sc")
                            nc.tensor.matmul(
                                sc[:],
                                lhsT=kt_sb[rows, sk * 128:(sk + 1) * 128],
                                rhs=qt_sb[rows, qcol + n * W: qcol + (n + 1) * W],
                                start=True, stop=True,
                            )
                            et = expp.tile([128, W], BF16, name="et")
                            dset = DVE_HALF_LAST if (blk == 1 and hl == 3) else DVE_HALF
                            if sk * 2 + n in dset:
                                nc.vector.tensor_scalar(
                                    et[:].bitcast(I16), sc[:], EXP_A, EXP_B, MUL, ADD
                                )
                            else:
                                nc.scalar.activation(et[:], sc[:], EXP, scale=0.125)
                            ets.append(et)
                        exp_tiles[sk] = ets
                        # interleave the previous block's out projection into
                        # this block's first head so PE never waits on the
                        # ht transposes at the block boundary
                        if blk == 1 and hl == 0 and sk < 8:
                            emit_out_chunk(0, sk)
                        if sk >= 2:
                            emit_pv(sk - 2)
                    emit_pv(NSK - 2)
                    emit_pv(NSK - 1)

                    # normalize + pack head pair [sq, h_even|h_odd] in bf16
                    if half == 0:
                        hn_t = hnp.tile([128, 8 * 128], BF16, name="hn")
                    rec = smalls.tile([128, 8], F32, name="rec")
                    for i in range(2):
                        nc.vector.reciprocal(
                            rec[:, i * 4:(i + 1) * 4], h_ps[i][:, 64:260:65]
                        )
                        h3 = h_ps[i][:, 0:260].rearrange(
                            "p (t j) -> p t j", t=4)[:, :, 0:64]
                        o3 = hn_t[:, i * W:(i + 1) * W].rearrange(
                            "p (t j) -> p t j", t=4)[:, :, half * 64: half * 64 + 64]
                        r3 = rec[:, i * 4:(i + 1) * 4].unsqueeze(2).broadcast_to([128, 4, 64])
                        nc.vector.tensor_tensor(out=o3, in0=h3, in1=r3, op=MUL)
                        if half == 1:
                            for t in range(4 * i, 4 * i + 4):
                                nc.sync.dma_start_transpose(
                                    ht_sb[:, pair * BLK + t * 128: pair * BLK + (t + 1) * 128],
                                    hn_t[:, t * 128:(t + 1) * 128],
                                )

                if blk == 1:
                    for s in range(8):
                        emit_out_chunk(1, s, tail=True)
                # block 0's out projection is emitted inside block 1 head 0

    nc.finalize()
    return nc


def _get_nc():
    if "nc" not in _CACHE:
        _CACHE["nc"] = _build_nc()
    return _CACHE["nc"]


def _to_bf16_bits(a):
    """fp32 -> bf16 bits (round-to-nearest-even), as uint16."""
    u = np.asarray(a, np.float32).view(np.uint32)
    return ((u + 0x7FFF + ((u >> 16) & 1)) >> 16).astype(np.uint16)


def _prep_core_inputs(inputs, wq, bq, wk, wv, wo):
    """Host-side shard prep: per-core transposed/rearranged operands."""
    xT = [np.ascontiguousarray(np.asarray(inputs[b], np.float32).T) for b in range(B)]
    wq3 = np.asarray(wq, np.float32).reshape(Dh, NUM_HEADS, D)
    bq2 = np.asarray(bq, np.float32).reshape(Dh, NUM_HEADS)
    wvkT = np.ascontiguousarray(
        np.concatenate([np.asarray(wk, np.float32).T, np.asarray(wv, np.float32).T], axis=1)
    )  # [1024, 128], K first
    wo_ = np.asarray(wo, np.float32)

    in_maps = []
    for c in range(N_CORES):
        b, g = divmod(c, G)
        heads = [g * HG + hl for hl in range(HG)]
        wqT_g = np.ascontiguousarray(
            np.concatenate([wq3[:, h, :].T for h in heads], axis=1)
        )  # [1024, 256]
        bq_g = np.ascontiguousarray(
            np.concatenate([bq2[:, h] for h in heads]).reshape(2, 128).T
        )  # [128, 2]: col p = heads (2p, 2p+1) stacked
        woT_g = _to_bf16_bits(wo_[:, g * QD:(g + 1) * QD].T)  # [256, 1024] bf16
        in_maps.append({
            "xT": xT[b],
            "wqT": wqT_g,
            "wvkT": wvkT,
            "woT": woT_g,
            "bq": bq_g,
        })
    return in_maps


def kernel(inputs, wq, bq, wk, bk, wv, bv, wo, bo):
    from concourse.bass_utils import run_bass_kernel_spmd

    nc = _get_nc()
    in_maps = _prep_core_inputs(inputs, wq, bq, wk, wv, wo)
    res = run_bass_kernel_spmd(nc, in_maps, list(range(N_CORES))).results

    wo_ = np.asarray(wo, np.float32)
    bias = (
        np.asarray(bo, np.float32)
        + wo_ @ np.tile(np.asarray(bv, np.float32), NUM_HEADS)
    )
    out = np.empty((B, S, D), np.float32)
    for b in range(B):
        acc = res[b * G]["part"].astype(np.float32).copy()
        for g in range(1, G):
            acc += res[b * G + g]["part"]
        out[b] = acc + bias
    return out
